# revision 10
# baseline (speedup 1.0000x reference)
"""Distributed Bass kernel for gated-adapter attention (head-sharded TP).

Layout strategy (per core, n_local = H/n_cores heads):
  stage A: QKV projections from xT (replicated input, [D, B*S+L] with adapter
           tokens appended as extra columns). Weights pre-transposed on host;
           wq/wk rows even/odd-permuted per head so RoPE becomes contiguous
           half-tile DVE ops. Q^T/K^T stored [128(d), n_local, BT] bf16
           (d-major), V stored [128(tok), n_vt, d_local] bf16 (token-major).
  stage B: causal attention with scores computed transposed (S^T[k, q]),
           no max-subtraction softmax (scores are small), two-segment
           softmax: local keys + adapter keys gated by tanh(gate).
           Normalizers via ones-matmul broadcast in PSUM.
  stage C: AllToAll redistributes per-head outputs to per-token-chunk,
           then output projection with full wo on each core's token chunk.

build_attn(chain=N) emits the whole body N times into one NEFF so a single
dispatch runs N back-to-back executions (used by test.py to measure
per-execution HW time without the ~10ms host dispatch overhead).
"""

import math
import numpy as np

from concourse import bass, bacc, tile
from concourse.tile_rust import add_dep_helper
from concourse import mybir

F32 = mybir.dt.float32
F32R = mybir.dt.float32r
BF16 = mybir.dt.bfloat16
AF = mybir.ActivationFunctionType
OP = mybir.AluOpType


def _emit_pass(nc, tc, dram, P, rep, timeline, stages, probes):
    ablate = set(filter(None, P.get("ablate", "").split(",")))
    """Emit one full attention pass (stages A-C) into the graph."""
    (n_cores, B, S, D, H, L, TBLK, QBLK, HD) = (
        P["n_cores"], P["B"], P["S"], P["D"], P["H"], P["L"],
        P["TBLK"], P["QBLK"], P["HD"])
    n_local = H // n_cores
    d_local = n_local * HD
    BT = B * S + L
    NT = B * S
    nkt = D // 128
    n_vt = (BT + 127) // 128
    n_jb = S // QBLK
    ndiag = QBLK // 128
    nn = D // 512
    inv_sqrt_hd = 1.0 / math.sqrt(HD)
    xt, wqt, wkt, wvt, wot, csa, csb, cmask, gate2, y_out = (
        P["xt"], P["wqt"], P["wkt"], P["wvt"], P["wot"], P["csa"],
        P["csb"], P["cmask"], P["gate2"], P["y"])
    sfx = f"_{rep}"

    blocks = []
    c0 = 0
    while c0 < BT:
        blocks.append((c0, min(TBLK, BT - c0)))
        c0 += min(TBLK, BT - c0)
    jp_pairs = [tuple(j for j in (a, a + 1) if j < n_jb)
                for a in range(0, n_jb, 2)]

    with tc.tile_pool(name="res" + sfx, bufs=1) as res:
        # ---- resident tensors ----
        qt_s = res.tile([128, n_local, BT], BF16)
        kt_s = res.tile([128, n_local, BT], BF16)
        v_s = res.tile([128, n_vt, d_local], BF16)
        cm_s = res.tile([128, QBLK + (ndiag - 1) * 128], BF16)
        ones128 = res.tile([128, 128], BF16)
        ones_l = res.tile([L, 128], BF16)
        avs = res.tile([L, d_local], BF16)

        nc.gpsimd.memset(ones128[:], 1.0)
        nc.gpsimd.memset(ones_l[:], 1.0)

        TPC2 = S // n_cores
        otl_b = [dram.tile([n_cores * d_local, TPC2], BF16,
                           name=f"otl{b}{sfx}") for b in range(B)]
        ogc_b = [dram.tile([n_cores * d_local, TPC2], BF16,
                           name=f"ogc{b}{sfx}") for b in range(B)]

        # ---- weight prep (QKV, kept fp32 -> used as fp32r) ----
        wqp = tc.alloc_tile_pool(name="wqp" + sfx, bufs=1)
        with tc.tile_pool(name="wprep" + sfx, bufs=2) as wprep:
            wq_r = wqp.tile([128, nkt, d_local], BF16)
            wk_r = wqp.tile([128, nkt, d_local], BF16)
            wv_r = wqp.tile([128, nkt, d_local], BF16)
            for src, dst in ((wqt, wq_r), (wkt, wk_r), (wvt, wv_r)):
                for kt in range(nkt):
                    eng = nc.scalar if kt % 2 == 0 else nc.gpsimd
                    eng.dma_start(dst[:, kt, :],
                                  src[kt * 128:(kt + 1) * 128, :])
            cstg = wprep.tile([128, QBLK + (ndiag - 1) * 128], F32,
                              tag="cstg")
            nc.sync.dma_start(cstg[:], cmask[:, :])
            nc.vector.tensor_copy(cm_s[:], cstg[:])

        # ---- stage A: QKV projections + RoPE ----
        with (
            tc.tile_pool(name="xp" + sfx, bufs=2) as xp,
            tc.tile_pool(name="rp" + sfx, bufs=2) as rp,
            tc.tile_pool(name="csp" + sfx, bufs=1) as csp,
            tc.tile_pool(name="psA" + sfx, bufs=5, space="PSUM") as psA,
            tc.tile_pool(name="psV" + sfx, bufs=3, space="PSUM") as psV,
        ):
            cs_a = csp.tile([128, S], F32)
            cs_b = csp.tile([128, S], F32)
            nc.sync.dma_start(cs_a[:], csa[:, :])
            nc.sync.dma_start(cs_b[:], csb[:, :])
            xbf_once = None
            for (c0, w) in (blocks if "A" in stages else []):
                is_adapter = c0 >= NT
                if "xdma" in ablate:
                    if xbf_once is None:
                        xbf_once = xp.tile([128, nkt, TBLK], BF16, tag="xbf")
                        for kt in range(nkt):
                            eng = nc.sync if kt % 2 == 0 else nc.gpsimd
                            eng.dma_start(xbf_once[:, kt, :w],
                                          xt[kt * 128:(kt + 1) * 128,
                                             c0:c0 + w])
                    xbf = xbf_once
                else:
                    xbf = xp.tile([128, nkt, TBLK], BF16, tag="xbf")
                    for kt in range(nkt):
                        eng = nc.sync if kt % 2 == 0 else nc.gpsimd
                        eng.dma_start(xbf[:, kt, :w],
                                      xt[kt * 128:(kt + 1) * 128,
                                         c0:c0 + w])

                s0 = c0 % S
                for wbf, dest in (() if "qk" in ablate else
                                  ((wq_r, qt_s), (wk_r, kt_s))):
                    for m in range(n_local):
                        ps = psA.tile([128, TBLK], F32, tag="qk")
                        for kt in range(nkt):
                            nc.tensor.matmul(
                                ps[:, :w],
                                lhsT=wbf[:, kt, m * 128:(m + 1) * 128],
                                rhs=xbf[:, kt, :w],
                                start=(kt == 0), stop=(kt == nkt - 1))
                        dcol = dest[:, m, c0:c0 + w]
                        if is_adapter or "rope" in ablate:
                            nc.scalar.copy(dcol, ps[:, :w])
                        else:
                            ca = cs_a[:, s0:s0 + w]
                            cb = cs_b[:, s0:s0 + w]
                            t1 = rp.tile([64, TBLK], F32, tag="t1")
                            t2 = rp.tile([64, TBLK], F32, tag="t2")
                            t3 = rp.tile([64, TBLK], F32, tag="t3")
                            t4 = rp.tile([64, TBLK], F32, tag="t4")
                            nc.vector.tensor_tensor(
                                t1[:, :w], ps[0:64, :w], ca[0:64, :], OP.mult)
                            nc.vector.tensor_tensor(
                                t2[:, :w], ps[64:128, :w], ca[64:128, :], OP.mult)
                            nc.vector.tensor_tensor(
                                dest[0:64, m, c0:c0 + w],
                                t1[:, :w], t2[:, :w], OP.subtract)
                            nc.vector.tensor_tensor(
                                t3[:, :w], ps[0:64, :w], cb[0:64, :], OP.mult)
                            nc.vector.tensor_tensor(
                                t4[:, :w], ps[64:128, :w], cb[64:128, :], OP.mult)
                            nc.vector.tensor_tensor(
                                dest[64:128, m, c0:c0 + w],
                                t3[:, :w], t4[:, :w], OP.add)
                for tt in ([] if ("v" in ablate and not is_adapter)
                           else range((w + 127) // 128)):
                    pw = min(128, w - tt * 128)
                    pv = psV.tile([128, d_local], F32, tag="v")
                    for kt in range(nkt):
                        nc.tensor.matmul(
                            pv[:pw, :],
                            lhsT=xbf[:, kt, tt * 128:tt * 128 + pw],
                            rhs=wv_r[:, kt, :],
                            start=(kt == 0), stop=(kt == nkt - 1))
                    nc.scalar.copy(v_s[:pw, c0 // 128 + tt, :], pv[:pw, :])

        wqp.release()

        # ---- gate: tanh + adapter-V scaling ----
        with tc.tile_pool(name="gp" + sfx, bufs=1) as gp:
            gsb = gp.tile([1, n_local], F32)
            nc.sync.dma_start(gsb[:], gate2[:, :])
            tgh = gp.tile([1, n_local], F32)
            nc.scalar.activation(tgh[:], gsb[:], AF.Tanh)
            att = NT // 128
            for h in range(n_local):
                tb = gp.tile([L, 1], F32, tag="tghb")
                nc.gpsimd.partition_broadcast(tb[:], tgh[0:1, h:h + 1])
                nc.vector.tensor_scalar(
                    avs[:, h * 128:(h + 1) * 128],
                    v_s[0:L, att, h * 128:(h + 1) * 128],
                    tb[:], None, OP.mult)

        # ---- wot prefetch (overlaps stage B; gpsimd dma casts f32->bf16)
        wotp = tc.alloc_tile_pool(name="wotp" + sfx, bufs=1)
        wot_bf = wotp.tile([128, nkt, D], BF16)
        for kt in range(nkt):
            nc.gpsimd.dma_start(wot_bf[:, kt, :],
                                wot[kt * 128:(kt + 1) * 128, :])

        # ---- stage B: attention ----
        cp = tc.alloc_tile_pool(name="cp" + sfx, bufs=2)
        og_sb = [cp.tile([128, nkt, S // n_cores], BF16, bufs=1,
                         name=f"ogsb{b}{sfx}") for b in range(B)]
        with (
            tc.tile_pool(name="ep" + sfx, bufs=4) as ep,
            tc.tile_pool(name="sp" + sfx, bufs=2) as sp,
            tc.tile_pool(name="psO" + sfx, bufs=2, space="PSUM") as psO,
            tc.tile_pool(name="psN" + sfx, bufs=2, space="PSUM") as psN,
            tc.tile_pool(name="psW" + sfx, bufs=4, space="PSUM") as psW,
        ):
            for b in (range(B) if "B" in stages else []):
                otl_writes = []
                for h in range(n_local):
                    for jp in jp_pairs:
                        out_ps = {}
                        norm_ps = {}
                        maxkt = {}
                        for j in jp:
                            out_ps[j] = psO.tile([128, QBLK], F32, tag="outp",
                                                 name=f"outp{j}{sfx}")
                            norm_ps[j] = psN.tile([128, QBLK], F32, tag="normp",
                                                  name=f"normp{j}{sfx}")
                            maxkt[j] = (j + 1) * ndiag
                        for kt in range(max(maxkt.values())):
                            for j in jp:
                                if kt >= maxkt[j]:
                                    continue
                                qsl = qt_s[:, h, b * S + j * QBLK:
                                           b * S + (j + 1) * QBLK]
                                sps = psW.tile([128, QBLK], F32, tag="w")
                                nc.tensor.matmul(
                                    sps[:],
                                    lhsT=kt_s[:, h, b * S + kt * 128:
                                              b * S + (kt + 1) * 128],
                                    rhs=qsl, start=True, stop=True)
                                e = ep.tile([128, QBLK], BF16, tag="e")
                                nc.scalar.activation(e[:], sps[:], AF.Exp,
                                                     scale=inv_sqrt_hd)
                                dk = kt - j * ndiag
                                if dk >= 0:
                                    st = (ndiag - 1 - dk) * 128
                                    nc.vector.tensor_tensor(
                                        e[:], e[:], cm_s[:, st:st + QBLK],
                                        OP.mult)
                                nc.tensor.matmul(
                                    norm_ps[j][:], lhsT=ones128[:], rhs=e[:],
                                    start=(kt == 0), stop=(kt == maxkt[j] - 1))
                                tt = (b * S) // 128 + kt
                                nc.tensor.matmul(
                                    out_ps[j][:],
                                    lhsT=v_s[:, tt, h * 128:(h + 1) * 128],
                                    rhs=e[:],
                                    start=(kt == 0), stop=(kt == maxkt[j] - 1))
                        for j in jp:
                            qsl = qt_s[:, h, b * S + j * QBLK:
                                       b * S + (j + 1) * QBLK]
                            sa = psW.tile([L, QBLK], F32, tag="w")
                            nc.tensor.matmul(
                                sa[:], lhsT=kt_s[:, h, NT:NT + L],
                                rhs=qsl, start=True, stop=True)
                            ea = ep.tile([L, QBLK], BF16, tag="ea")
                            nc.scalar.activation(ea[:], sa[:], AF.Exp,
                                                 scale=inv_sqrt_hd)
                            na = psW.tile([128, QBLK], F32, tag="w")
                            nc.tensor.matmul(na[:], lhsT=ones_l[:],
                                             rhs=ea[:], start=True, stop=True)
                            rca = sp.tile([L, QBLK], F32, tag="rca")
                            nc.vector.reciprocal(rca[:], na[0:L, :])
                            pa = ep.tile([L, QBLK], BF16, tag="ea")
                            nc.vector.tensor_tensor(pa[:], ea[:], rca[:],
                                                    OP.mult)
                            oa = psW.tile([128, QBLK], F32, tag="w")
                            nc.tensor.matmul(
                                oa[:], lhsT=avs[:, h * 128:(h + 1) * 128],
                                rhs=pa[:], start=True, stop=True)
                            rcl = sp.tile([128, QBLK], F32, tag="rcl")
                            nc.vector.reciprocal(rcl[:], norm_ps[j][:])
                            tt1 = sp.tile([128, QBLK], F32, tag="t1c")
                            nc.vector.tensor_tensor(tt1[:], out_ps[j][:],
                                                    rcl[:], OP.mult)
                            fin = sp.tile([128, QBLK], BF16, tag="fin")
                            nc.vector.tensor_tensor(fin[:], tt1[:], oa[:],
                                                    OP.add)
                            for sc in range(QBLK // TPC2):
                                scol = j * QBLK + sc * TPC2
                                cidx = scol // TPC2
                                wi = nc.sync.dma_start(
                                    otl_b[b][cidx * d_local + h * 128:
                                             cidx * d_local + (h + 1) * 128,
                                             :],
                                    fin[:, sc * TPC2:(sc + 1) * TPC2])
                                otl_writes.append(wi)
                # per-batch AllToAll + og prefetch, overlaps next batch
                if timeline or P.get("coll_dma"):
                    cc = nc.sync.dma_start(ogc_b[b][:, :], otl_b[b][:, :])
                else:
                    cc = nc.gpsimd.collective_compute(
                        "AllToAll", OP.bypass,
                        replica_groups=[list(range(n_cores))],
                        ins=[otl_b[b].opt()], outs=[ogc_b[b].opt()])
                for wi in otl_writes:
                    add_dep_helper(cc.ins, wi.ins, sync=True,
                                   reason="a2a waits for otl writes")
                for kt in range(nkt):
                    eng = nc.gpsimd if kt % 2 == 0 else nc.sync
                    ld = eng.dma_start(og_sb[b][:, kt, :],
                                       ogc_b[b][kt * 128:(kt + 1) * 128, :])
                    add_dep_helper(ld.ins, cc.ins, sync=True,
                                   reason="og load waits for a2a")

        # ---- stage C: output projection per batch ----
        ntc2 = TPC2 // 128
        with tc.tile_pool(name="psY" + sfx, bufs=4, space="PSUM") as psY:
            for b in (range(B) if "C" in stages else []):
                for tc_i in range(ntc2):
                    pys = [psY.tile([128, 512], F32, tag="y",
                                    name=f"py{b}_{tc_i}_{n}{sfx}")
                           for n in range(nn)]
                    for kt in range(nkt):
                        for n in range(nn):
                            nc.tensor.matmul(
                                pys[n][:],
                                lhsT=og_sb[b][:, kt,
                                              tc_i * 128:(tc_i + 1) * 128],
                                rhs=wot_bf[:, kt, n * 512:(n + 1) * 512],
                                start=(kt == 0), stop=(kt == nkt - 1))
                    for n in range(nn):
                        ysb = cp.tile([128, 512], F32, tag="ysb")
                        nc.scalar.copy(ysb[:], pys[n][:])
                        nc.sync.dma_start(
                            y_out[b * TPC2 + tc_i * 128:
                                  b * TPC2 + (tc_i + 1) * 128,
                                  n * 512:(n + 1) * 512],
                            ysb[:])

        # ---- probes: tail DMA reads that pin tensor liveness to the end.
        # A fully stripped build deterministically miscompares on HW (the
        # allocator/scheduler takes a hazardous path); reading qt_s at the
        # end is empirically sufficient to steer it back. "internal*" writes
        # go to on-device DRAM scratch (no host transfer, ~zero cost).
        if probes and probes.startswith("internal"):
            sel = probes[len("internal"):] or "qkvwog"
            if "q" in sel:
                dbg_q = dram.tile([128, n_local * BT], BF16,
                                  name="dbg_q" + sfx)
                nc.sync.dma_start(dbg_q[:, :], qt_s.opt())
            if "k" in sel:
                dbg_k = dram.tile([128, n_local * BT], BF16,
                                  name="dbg_k" + sfx)
                nc.sync.dma_start(dbg_k[:, :], kt_s.opt())
            if "v" in sel:
                dbg_v = dram.tile([128, n_vt * d_local], BF16,
                                  name="dbg_v" + sfx)
                nc.sync.dma_start(dbg_v[:, :], v_s.opt())
            if "w" in sel:
                dbg_w = dram.tile([128, nkt * D], BF16, name="dbg_w" + sfx)
                nc.sync.dma_start(dbg_w[:, :], wot_bf.opt())
            if "o" in sel:
                dbg_o = dram.tile([n_cores * d_local, TPC2], BF16,
                                  name="dbg_o" + sfx)
                nc.sync.dma_start(dbg_o[:, :], otl_b[0][:, :])
            if "g" in sel:
                dbg_g = dram.tile([n_cores * d_local, TPC2], BF16,
                                  name="dbg_g" + sfx)
                nc.sync.dma_start(dbg_g[:, :], ogc_b[0][:, :])
        cp.release()
        wotp.release()


def build_attn(n_cores=8, B=2, S=2048, D=2048, H=16, L=10,
               TBLK=512, QBLK=512, HD=128, timeline=False, stages="ABC",
               probes=False, chain=1, coll_dma=False, ablate=""):
    if probes is True:
        probes = "external"
    assert probes != "external", "external probes removed; use internal*"
    n_local = H // n_cores
    d_local = n_local * HD
    BT = B * S + L
    NT = B * S
    TPC = NT // n_cores
    nkt = D // 128
    assert NT % 128 == 0 and S % TBLK == 0 and QBLK % 128 == 0
    assert HD == 128, "layout assumes head_dim == 128"

    nc = bacc.Bacc(None, num_devices=(1 if timeline else n_cores),
                   debug=False)

    P = dict(n_cores=n_cores, B=B, S=S, D=D, H=H, L=L, TBLK=TBLK,
             QBLK=QBLK, HD=HD, coll_dma=coll_dma, ablate=ablate)
    P["xt"] = nc.declare_dram_parameter("xt", [D, BT], BF16, False)
    P["wqt"] = nc.declare_dram_parameter("wqt", [D, d_local], BF16, False)
    P["wkt"] = nc.declare_dram_parameter("wkt", [D, d_local], BF16, False)
    P["wvt"] = nc.declare_dram_parameter("wvt", [D, d_local], BF16, False)
    P["wot"] = nc.declare_dram_parameter("wot", [D, D], BF16, False)
    P["csa"] = nc.declare_dram_parameter("csa", [128, S], F32, False)
    P["csb"] = nc.declare_dram_parameter("csb", [128, S], F32, False)
    ndiag = QBLK // 128
    P["cmask"] = nc.declare_dram_parameter(
        "cmask", [128, QBLK + (ndiag - 1) * 128], F32, False)
    P["gate2"] = nc.declare_dram_parameter("gate2", [1, n_local], F32, False)
    P["y"] = nc.declare_dram_parameter("y", [TPC, D], F32, True)

    with tile.TileContext(nc) as tc:
        with tc.tile_pool(name="dram", bufs=1, space="DRAM") as dram:
            for rep in range(chain):
                _emit_pass(nc, tc, dram, P, rep, timeline, stages, probes)

    nc.compile()
    return nc


def make_in_maps(x, adapter, wq, wk, wv, wo, gate, freqs_cis,
                 n_cores=8, B=2, S=2048, D=2048, H=16, L=10, QBLK=512):
    """Host-side sharding/layout prep. Returns list of per-core input dicts."""
    HD = 128
    n_local = H // n_cores
    d_local = n_local * HD
    x = np.asarray(x, np.float32)
    adapter = np.asarray(adapter, np.float32)
    ndiag = QBLK // 128

    import ml_dtypes
    bf16 = ml_dtypes.bfloat16
    xt = np.concatenate([x[b].T for b in range(B)] + [adapter[0].T], axis=1)
    xt_bf = np.ascontiguousarray(xt, np.float32).astype(bf16)
    wot_bf = np.ascontiguousarray(np.asarray(wo, np.float32).T).astype(bf16)

    fc = np.asarray(freqs_cis, np.float32)
    cos = np.ascontiguousarray(fc[:, :, 0].T)
    sin = np.ascontiguousarray(fc[:, :, 1].T)
    csa = np.concatenate([cos, sin], axis=0)
    csb = np.concatenate([sin, cos], axis=0)

    W = QBLK + (ndiag - 1) * 128
    p = np.arange(128)[:, None]
    v = np.arange(W)[None, :]
    cmask = ((v - (ndiag - 1) * 128) >= p).astype(np.float32)

    ev = np.arange(0, HD, 2)
    od = np.arange(1, HD, 2)
    head_perm = np.concatenate([ev, od])

    gate_f = np.asarray(gate, np.float32).reshape(H)
    in_maps = []
    for c in range(n_cores):
        rows = []
        for hl in range(n_local):
            hg = c * n_local + hl
            rows.append(hg * HD + head_perm)
        rows_p = np.concatenate(rows)
        rows_n = np.concatenate([np.arange(hg * HD, (hg + 1) * HD)
                                 for hg in range(c * n_local,
                                                 (c + 1) * n_local)])
        in_maps.append({
            "xt": xt_bf,
            "wqt": np.ascontiguousarray(wq[rows_p, :].T).astype(bf16),
            "wkt": np.ascontiguousarray(wk[rows_p, :].T).astype(bf16),
            "wvt": np.ascontiguousarray(wv[rows_n, :].T).astype(bf16),
            "wot": wot_bf,
            "csa": csa, "csb": csb, "cmask": cmask,
            "gate2": gate_f[c * n_local:(c + 1) * n_local].reshape(1, n_local),
        })
    return in_maps


def assemble_output(results, n_cores=8, B=2, S=2048, D=2048):
    TPC2 = S // n_cores
    y = np.zeros((B, S, D), np.float32)
    for c in range(n_cores):
        yc = results[c]["y"]
        for b in range(B):
            y[b, TPC2 * c:TPC2 * (c + 1), :] = yc[b * TPC2:(b + 1) * TPC2]
    return y


# ---------------------------------------------------------------------------
# Harness entry point: takes FULL inputs, returns FULL output.
# ---------------------------------------------------------------------------

def kernel(x, adapter, wq, wk, wv, wo, gate, freqs_cis, mask):
    """Gated-adapter attention on 8 TRN2 NeuronCores (head-sharded TP).

    probes="internalq" keeps one tail probe DMA of qt_s in the graph (its
    end-of-graph read pins the tensor's SBUF liveness, which steers the
    allocator/scheduler away from a hazard that a fully stripped build
    deterministically exposes) but writes it to on-device DRAM scratch
    instead of an ExternalOutput, so it costs no host transfer.
    """
    from concourse.bass_utils import run_bass_kernel_spmd

    n_cores = 8
    nc = build_attn(n_cores=n_cores, probes="internalq")
    in_maps = make_in_maps(x, adapter, wq, wk, wv, wo, gate, freqs_cis,
                           n_cores=n_cores)
    r = run_bass_kernel_spmd(nc, in_maps, core_ids=list(range(n_cores)))
    return assemble_output(r.results, n_cores=n_cores)



# revision 16
# speedup vs baseline: 1.2748x; 1.2748x over previous
"""Distributed Bass kernel for gated-adapter attention (head-sharded TP).

Layout strategy (per core, n_local = H/n_cores heads):
  stage A: QKV projections from xT (replicated input, [D, B*S+L] with adapter
           tokens appended as extra columns). Weights pre-transposed on host;
           wq/wk rows even/odd-permuted per head so RoPE becomes contiguous
           half-tile DVE ops. Q^T/K^T stored [128(d), n_local, BT] bf16
           (d-major), V stored [128(tok), n_vt, d_local] bf16 (token-major).
  stage B: causal attention with scores computed transposed (S^T[k, q]),
           no max-subtraction softmax (scores are small), two-segment
           softmax: local keys + adapter keys gated by tanh(gate).
           Normalizers via ones-matmul broadcast in PSUM.
  stage C: AllToAll redistributes per-head outputs to per-token-chunk,
           then output projection with full wo on each core's token chunk.

build_attn(chain=N) emits the whole body N times into one NEFF so a single
dispatch runs N back-to-back executions (used by test.py to measure
per-execution HW time without the ~10ms host dispatch overhead).
"""

import math
import numpy as np

from concourse import bass, bacc, tile
from concourse.tile_rust import add_dep_helper
from concourse import mybir

F32 = mybir.dt.float32
F32R = mybir.dt.float32r
BF16 = mybir.dt.bfloat16
AF = mybir.ActivationFunctionType
OP = mybir.AluOpType


def _emit_pass(nc, tc, dram, P, rep, timeline, stages, probes):
    ablate = set(filter(None, P.get("ablate", "").split(",")))
    """Emit one full attention pass (stages A-C) into the graph."""
    (n_cores, B, S, D, H, L, TBLK, QBLK, HD) = (
        P["n_cores"], P["B"], P["S"], P["D"], P["H"], P["L"],
        P["TBLK"], P["QBLK"], P["HD"])
    n_local = H // n_cores
    d_local = n_local * HD
    BT = B * S + L
    NT = B * S
    nkt = D // 128
    n_vt = (BT + 127) // 128
    n_jb = S // QBLK
    ndiag = QBLK // 128
    nn = D // 512
    inv_sqrt_hd = 1.0 / math.sqrt(HD)
    xt, wqt, wkt, wvt, wot, csa, csb, cmask, gate2, y_out = (
        P["xt"], P["wqt"], P["wkt"], P["wvt"], P["wot"], P["csa"],
        P["csb"], P["cmask"], P["gate2"], P["y"])
    sfx = f"_{rep}"

    blocks = []
    c0 = 0
    while c0 < BT:
        blocks.append((c0, min(TBLK, BT - c0)))
        c0 += min(TBLK, BT - c0)
    jp_pairs = [tuple(j for j in (a, a + 1) if j < n_jb)
                for a in range(0, n_jb, 2)]

    with tc.tile_pool(name="res" + sfx, bufs=1) as res:
        # ---- resident tensors ----
        qt_s = res.tile([128, n_local, BT], BF16)
        kt_s = res.tile([128, n_local, BT], BF16)
        v_s = res.tile([128, n_vt, d_local], BF16)
        cm_s = res.tile([128, QBLK + (ndiag - 1) * 128], BF16)
        ones128 = res.tile([128, 128], BF16)
        ones_l = res.tile([L, 128], BF16)
        avs = res.tile([L, d_local], BF16)

        nc.gpsimd.memset(ones128[:], 1.0)
        nc.gpsimd.memset(ones_l[:], 1.0)

        TPC2 = S // n_cores
        otl_b = [dram.tile([n_cores * d_local, TPC2], BF16,
                           name=f"otl{b}{sfx}") for b in range(B)]
        ogc_b = [dram.tile([n_cores * d_local, TPC2], BF16,
                           name=f"ogc{b}{sfx}") for b in range(B)]

        # ---- weight prep: single batched DMA per tensor (host pre-laid
        # [128, nkt*d_local] partition-major so one dma_start moves it all)
        wqp = tc.alloc_tile_pool(name="wqp" + sfx, bufs=1)
        wq_r = wqp.tile([128, nkt, d_local], BF16)
        wk_r = wqp.tile([128, nkt, d_local], BF16)
        wv_r = wqp.tile([128, nkt, d_local], BF16)
        nc.scalar.dma_start(wq_r[:, :, :], wqt[:, :, :])
        nc.scalar.dma_start(wk_r[:, :, :], wkt[:, :, :])
        nc.scalar.dma_start(wv_r[:, :, :], wvt[:, :, :])
        nc.gpsimd.dma_start(cm_s[:], cmask[:, :])

        # ---- stage A: QKV projections + RoPE ----
        with (
            tc.tile_pool(name="xp" + sfx, bufs=2) as xp,
            tc.tile_pool(name="rp" + sfx, bufs=2) as rp,
            tc.tile_pool(name="csp" + sfx, bufs=1) as csp,
            tc.tile_pool(name="psA" + sfx, bufs=5, space="PSUM") as psA,
            tc.tile_pool(name="psV" + sfx, bufs=3, space="PSUM") as psV,
        ):
            cs_a = csp.tile([128, S], F32)
            cs_b = csp.tile([128, S], F32)
            nc.gpsimd.dma_start(cs_a[:], csa[:, :])
            nc.gpsimd.dma_start(cs_b[:], csb[:, :])
            for bi, (c0, w) in enumerate(blocks if "A" in stages else []):
                is_adapter = c0 >= NT
                # one DMA per block: host laid xt block-major
                # [128, nblk, nkt, TBLK] so the whole [128, nkt*TBLK]
                # tile is one contiguous-per-partition transfer
                xbf = xp.tile([128, nkt, TBLK], BF16, tag="xbf")
                nc.sync.dma_start(xbf[:, :, :], xt[:, bi, :, :])

                s0 = c0 % S
                for wbf, dest in (() if "qk" in ablate else
                                  ((wq_r, qt_s), (wk_r, kt_s))):
                    for m in range(n_local):
                        ps = psA.tile([128, TBLK], F32, tag="qk")
                        for kt in range(nkt):
                            nc.tensor.matmul(
                                ps[:, :w],
                                lhsT=wbf[:, kt, m * 128:(m + 1) * 128],
                                rhs=xbf[:, kt, :w],
                                start=(kt == 0), stop=(kt == nkt - 1))
                        dcol = dest[:, m, c0:c0 + w]
                        if is_adapter or "rope" in ablate:
                            nc.scalar.copy(dcol, ps[:, :w])
                        else:
                            ca = cs_a[:, s0:s0 + w]
                            cb = cs_b[:, s0:s0 + w]
                            t1 = rp.tile([64, TBLK], F32, tag="t1")
                            t2 = rp.tile([64, TBLK], F32, tag="t2")
                            t3 = rp.tile([64, TBLK], F32, tag="t3")
                            t4 = rp.tile([64, TBLK], F32, tag="t4")
                            nc.vector.tensor_tensor(
                                t1[:, :w], ps[0:64, :w], ca[0:64, :], OP.mult)
                            nc.vector.tensor_tensor(
                                t2[:, :w], ps[64:128, :w], ca[64:128, :], OP.mult)
                            nc.vector.tensor_tensor(
                                dest[0:64, m, c0:c0 + w],
                                t1[:, :w], t2[:, :w], OP.subtract)
                            nc.vector.tensor_tensor(
                                t3[:, :w], ps[0:64, :w], cb[0:64, :], OP.mult)
                            nc.vector.tensor_tensor(
                                t4[:, :w], ps[64:128, :w], cb[64:128, :], OP.mult)
                            nc.vector.tensor_tensor(
                                dest[64:128, m, c0:c0 + w],
                                t3[:, :w], t4[:, :w], OP.add)
                for tt in ([] if ("v" in ablate and not is_adapter)
                           else range((w + 127) // 128)):
                    pw = min(128, w - tt * 128)
                    pv = psV.tile([128, d_local], F32, tag="v")
                    for kt in range(nkt):
                        nc.tensor.matmul(
                            pv[:pw, :],
                            lhsT=xbf[:, kt, tt * 128:tt * 128 + pw],
                            rhs=wv_r[:, kt, :],
                            start=(kt == 0), stop=(kt == nkt - 1))
                    nc.scalar.copy(v_s[:pw, c0 // 128 + tt, :], pv[:pw, :])

        wqp.release()

        # ---- gate: tanh + adapter-V scaling ----
        with tc.tile_pool(name="gp" + sfx, bufs=1) as gp:
            gsb = gp.tile([1, n_local], F32)
            nc.sync.dma_start(gsb[:], gate2[:, :])
            tgh = gp.tile([1, n_local], F32)
            nc.scalar.activation(tgh[:], gsb[:], AF.Tanh)
            att = NT // 128
            for h in range(n_local):
                tb = gp.tile([L, 1], F32, tag="tghb")
                nc.gpsimd.partition_broadcast(tb[:], tgh[0:1, h:h + 1])
                nc.vector.tensor_scalar(
                    avs[:, h * 128:(h + 1) * 128],
                    v_s[0:L, att, h * 128:(h + 1) * 128],
                    tb[:], None, OP.mult)

        # ---- wot prefetch (overlaps stage B; one 8MB batched DMA)
        wotp = tc.alloc_tile_pool(name="wotp" + sfx, bufs=1)
        wot_bf = wotp.tile([128, nkt, D], BF16)
        nc.scalar.dma_start(wot_bf[:, :, :], wot[:, :, :])

        # ---- stage B: attention ----
        cp = tc.alloc_tile_pool(name="cp" + sfx, bufs=2)
        og_sb = [cp.tile([128, nkt, S // n_cores], BF16, bufs=1,
                         name=f"ogsb{b}{sfx}") for b in range(B)]
        with (
            tc.tile_pool(name="ep" + sfx, bufs=4) as ep,
            tc.tile_pool(name="sp" + sfx, bufs=2) as sp,
            tc.tile_pool(name="psO" + sfx, bufs=2, space="PSUM") as psO,
            tc.tile_pool(name="psN" + sfx, bufs=2, space="PSUM") as psN,
            tc.tile_pool(name="psW" + sfx, bufs=4, space="PSUM") as psW,
        ):
            for b in (range(B) if "B" in stages else []):
                otl_writes = []
                for h in range(n_local):
                    for jp in jp_pairs:
                        out_ps = {}
                        norm_ps = {}
                        maxkt = {}
                        for j in jp:
                            out_ps[j] = psO.tile([128, QBLK], F32, tag="outp",
                                                 name=f"outp{j}{sfx}")
                            norm_ps[j] = psN.tile([128, QBLK], F32, tag="normp",
                                                  name=f"normp{j}{sfx}")
                            maxkt[j] = (j + 1) * ndiag
                        for kt in range(max(maxkt.values())):
                            for j in jp:
                                if kt >= maxkt[j]:
                                    continue
                                qsl = qt_s[:, h, b * S + j * QBLK:
                                           b * S + (j + 1) * QBLK]
                                sps = psW.tile([128, QBLK], F32, tag="w")
                                nc.tensor.matmul(
                                    sps[:],
                                    lhsT=kt_s[:, h, b * S + kt * 128:
                                              b * S + (kt + 1) * 128],
                                    rhs=qsl, start=True, stop=True)
                                e = ep.tile([128, QBLK], BF16, tag="e")
                                nc.scalar.activation(e[:], sps[:], AF.Exp,
                                                     scale=inv_sqrt_hd)
                                dk = kt - j * ndiag
                                if dk >= 0:
                                    st = (ndiag - 1 - dk) * 128
                                    nc.vector.tensor_tensor(
                                        e[:], e[:], cm_s[:, st:st + QBLK],
                                        OP.mult)
                                nc.tensor.matmul(
                                    norm_ps[j][:], lhsT=ones128[:], rhs=e[:],
                                    start=(kt == 0), stop=(kt == maxkt[j] - 1))
                                tt = (b * S) // 128 + kt
                                nc.tensor.matmul(
                                    out_ps[j][:],
                                    lhsT=v_s[:, tt, h * 128:(h + 1) * 128],
                                    rhs=e[:],
                                    start=(kt == 0), stop=(kt == maxkt[j] - 1))
                        for j in jp:
                            qsl = qt_s[:, h, b * S + j * QBLK:
                                       b * S + (j + 1) * QBLK]
                            sa = psW.tile([L, QBLK], F32, tag="w")
                            nc.tensor.matmul(
                                sa[:], lhsT=kt_s[:, h, NT:NT + L],
                                rhs=qsl, start=True, stop=True)
                            ea = ep.tile([L, QBLK], BF16, tag="ea")
                            nc.scalar.activation(ea[:], sa[:], AF.Exp,
                                                 scale=inv_sqrt_hd)
                            na = psW.tile([128, QBLK], F32, tag="w")
                            nc.tensor.matmul(na[:], lhsT=ones_l[:],
                                             rhs=ea[:], start=True, stop=True)
                            rca = sp.tile([L, QBLK], F32, tag="rca")
                            nc.vector.reciprocal(rca[:], na[0:L, :])
                            pa = ep.tile([L, QBLK], BF16, tag="ea")
                            nc.vector.tensor_tensor(pa[:], ea[:], rca[:],
                                                    OP.mult)
                            oa = psW.tile([128, QBLK], F32, tag="w")
                            nc.tensor.matmul(
                                oa[:], lhsT=avs[:, h * 128:(h + 1) * 128],
                                rhs=pa[:], start=True, stop=True)
                            rcl = sp.tile([128, QBLK], F32, tag="rcl")
                            nc.vector.reciprocal(rcl[:], norm_ps[j][:])
                            tt1 = sp.tile([128, QBLK], F32, tag="t1c")
                            nc.vector.tensor_tensor(tt1[:], out_ps[j][:],
                                                    rcl[:], OP.mult)
                            fin = sp.tile([128, QBLK], BF16, tag="fin")
                            nc.vector.tensor_tensor(fin[:], tt1[:], oa[:],
                                                    OP.add)
                            for sc in range(QBLK // TPC2):
                                scol = j * QBLK + sc * TPC2
                                cidx = scol // TPC2
                                wi = nc.sync.dma_start(
                                    otl_b[b][cidx * d_local + h * 128:
                                             cidx * d_local + (h + 1) * 128,
                                             :],
                                    fin[:, sc * TPC2:(sc + 1) * TPC2])
                                otl_writes.append(wi)
                # per-batch AllToAll + og prefetch, overlaps next batch
                if timeline or P.get("coll_dma"):
                    cc = nc.sync.dma_start(ogc_b[b][:, :], otl_b[b][:, :])
                else:
                    cc = nc.gpsimd.collective_compute(
                        "AllToAll", OP.bypass,
                        replica_groups=[list(range(n_cores))],
                        ins=[otl_b[b].opt()], outs=[ogc_b[b].opt()])
                for wi in otl_writes:
                    add_dep_helper(cc.ins, wi.ins, sync=True,
                                   reason="a2a waits for otl writes")
                for kt in range(nkt):
                    eng = nc.gpsimd if kt % 2 == 0 else nc.sync
                    ld = eng.dma_start(og_sb[b][:, kt, :],
                                       ogc_b[b][kt * 128:(kt + 1) * 128, :])
                    add_dep_helper(ld.ins, cc.ins, sync=True,
                                   reason="og load waits for a2a")

        # ---- stage C: output projection per batch ----
        ntc2 = TPC2 // 128
        with tc.tile_pool(name="psY" + sfx, bufs=4, space="PSUM") as psY:
            for b in (range(B) if "C" in stages else []):
                for tc_i in range(ntc2):
                    pys = [psY.tile([128, 512], F32, tag="y",
                                    name=f"py{b}_{tc_i}_{n}{sfx}")
                           for n in range(nn)]
                    for kt in range(nkt):
                        for n in range(nn):
                            nc.tensor.matmul(
                                pys[n][:],
                                lhsT=og_sb[b][:, kt,
                                              tc_i * 128:(tc_i + 1) * 128],
                                rhs=wot_bf[:, kt, n * 512:(n + 1) * 512],
                                start=(kt == 0), stop=(kt == nkt - 1))
                    for n in range(nn):
                        ysb = cp.tile([128, 512], F32, tag="ysb")
                        nc.scalar.copy(ysb[:], pys[n][:])
                        nc.sync.dma_start(
                            y_out[b * TPC2 + tc_i * 128:
                                  b * TPC2 + (tc_i + 1) * 128,
                                  n * 512:(n + 1) * 512],
                            ysb[:])

        # ---- probes: tail DMA reads that pin tensor liveness to the end.
        # A fully stripped build deterministically miscompares on HW (the
        # allocator/scheduler takes a hazardous path); reading qt_s at the
        # end is empirically sufficient to steer it back. "internal*" writes
        # go to on-device DRAM scratch (no host transfer, ~zero cost).
        if probes and probes.startswith("internal"):
            sel = probes[len("internal"):] or "qkvwog"
            if "q" in sel:
                dbg_q = dram.tile([128, n_local * BT], BF16,
                                  name="dbg_q" + sfx)
                nc.sync.dma_start(dbg_q[:, :], qt_s.opt())
            if "k" in sel:
                dbg_k = dram.tile([128, n_local * BT], BF16,
                                  name="dbg_k" + sfx)
                nc.sync.dma_start(dbg_k[:, :], kt_s.opt())
            if "v" in sel:
                dbg_v = dram.tile([128, n_vt * d_local], BF16,
                                  name="dbg_v" + sfx)
                nc.sync.dma_start(dbg_v[:, :], v_s.opt())
            if "w" in sel:
                dbg_w = dram.tile([128, nkt * D], BF16, name="dbg_w" + sfx)
                nc.sync.dma_start(dbg_w[:, :], wot_bf.opt())
            if "o" in sel:
                dbg_o = dram.tile([n_cores * d_local, TPC2], BF16,
                                  name="dbg_o" + sfx)
                nc.sync.dma_start(dbg_o[:, :], otl_b[0][:, :])
            if "g" in sel:
                dbg_g = dram.tile([n_cores * d_local, TPC2], BF16,
                                  name="dbg_g" + sfx)
                nc.sync.dma_start(dbg_g[:, :], ogc_b[0][:, :])
        cp.release()
        wotp.release()


def build_attn(n_cores=8, B=2, S=2048, D=2048, H=16, L=10,
               TBLK=512, QBLK=512, HD=128, timeline=False, stages="ABC",
               probes=False, chain=1, coll_dma=False, ablate=""):
    if probes is True:
        probes = "external"
    assert probes != "external", "external probes removed; use internal*"
    n_local = H // n_cores
    d_local = n_local * HD
    BT = B * S + L
    NT = B * S
    TPC = NT // n_cores
    nkt = D // 128
    assert NT % 128 == 0 and S % TBLK == 0 and QBLK % 128 == 0
    assert HD == 128, "layout assumes head_dim == 128"

    nc = bacc.Bacc(None, num_devices=(1 if timeline else n_cores),
                   debug=False)

    P = dict(n_cores=n_cores, B=B, S=S, D=D, H=H, L=L, TBLK=TBLK,
             QBLK=QBLK, HD=HD, coll_dma=coll_dma, ablate=ablate)
    nblk = (BT + TBLK - 1) // TBLK
    nkt = D // 128
    P["xt"] = nc.declare_dram_parameter("xt", [128, nblk, nkt, TBLK],
                                        BF16, False)
    P["wqt"] = nc.declare_dram_parameter("wqt", [128, nkt, d_local],
                                         BF16, False)
    P["wkt"] = nc.declare_dram_parameter("wkt", [128, nkt, d_local],
                                         BF16, False)
    P["wvt"] = nc.declare_dram_parameter("wvt", [128, nkt, d_local],
                                         BF16, False)
    P["wot"] = nc.declare_dram_parameter("wot", [128, nkt, D], BF16, False)
    P["csa"] = nc.declare_dram_parameter("csa", [128, S], F32, False)
    P["csb"] = nc.declare_dram_parameter("csb", [128, S], F32, False)
    ndiag = QBLK // 128
    P["cmask"] = nc.declare_dram_parameter(
        "cmask", [128, QBLK + (ndiag - 1) * 128], BF16, False)
    P["gate2"] = nc.declare_dram_parameter("gate2", [1, n_local], F32, False)
    P["y"] = nc.declare_dram_parameter("y", [TPC, D], F32, True)

    with tile.TileContext(nc) as tc:
        with tc.tile_pool(name="dram", bufs=1, space="DRAM") as dram:
            for rep in range(chain):
                _emit_pass(nc, tc, dram, P, rep, timeline, stages, probes)

    nc.compile()
    return nc


def make_in_maps(x, adapter, wq, wk, wv, wo, gate, freqs_cis,
                 n_cores=8, B=2, S=2048, D=2048, H=16, L=10, QBLK=512):
    """Host-side sharding/layout prep. Returns list of per-core input dicts."""
    HD = 128
    n_local = H // n_cores
    d_local = n_local * HD
    x = np.asarray(x, np.float32)
    adapter = np.asarray(adapter, np.float32)
    ndiag = QBLK // 128

    import ml_dtypes
    bf16 = ml_dtypes.bfloat16
    D = x.shape[2]
    TBLK = 512
    nkt = D // 128
    BT = B * S + L
    nblk = (BT + TBLK - 1) // TBLK
    xt = np.concatenate([x[b].T for b in range(B)] + [adapter[0].T], axis=1)
    # pad token dim to nblk*TBLK, then lay out [128, nblk, nkt, TBLK] so
    # each block's SBUF tile is a single contiguous-per-partition DMA
    xt_pad = np.zeros((D, nblk * TBLK), np.float32)
    xt_pad[:, :BT] = xt
    xt_bf = np.ascontiguousarray(
        xt_pad.reshape(nkt, 128, nblk, TBLK).transpose(1, 2, 0, 3)
    ).astype(bf16)
    wot_bf = np.ascontiguousarray(
        np.asarray(wo, np.float32).T.reshape(nkt, 128, D).transpose(1, 0, 2)
    ).astype(bf16)

    fc = np.asarray(freqs_cis, np.float32)
    cos = np.ascontiguousarray(fc[:, :, 0].T)
    sin = np.ascontiguousarray(fc[:, :, 1].T)
    csa = np.concatenate([cos, sin], axis=0)
    csb = np.concatenate([sin, cos], axis=0)

    W = QBLK + (ndiag - 1) * 128
    p = np.arange(128)[:, None]
    v = np.arange(W)[None, :]
    cmask = ((v - (ndiag - 1) * 128) >= p).astype(bf16)

    ev = np.arange(0, HD, 2)
    od = np.arange(1, HD, 2)
    head_perm = np.concatenate([ev, od])

    gate_f = np.asarray(gate, np.float32).reshape(H)
    in_maps = []
    for c in range(n_cores):
        rows = []
        for hl in range(n_local):
            hg = c * n_local + hl
            rows.append(hg * HD + head_perm)
        rows_p = np.concatenate(rows)
        rows_n = np.concatenate([np.arange(hg * HD, (hg + 1) * HD)
                                 for hg in range(c * n_local,
                                                 (c + 1) * n_local)])
        def _wlay(w2):
            # [D, d_local] -> [128, nkt, d_local] partition-major
            return np.ascontiguousarray(
                w2.reshape(nkt, 128, d_local).transpose(1, 0, 2)
            ).astype(bf16)

        in_maps.append({
            "xt": xt_bf,
            "wqt": _wlay(np.asarray(wq[rows_p, :].T, np.float32)),
            "wkt": _wlay(np.asarray(wk[rows_p, :].T, np.float32)),
            "wvt": _wlay(np.asarray(wv[rows_n, :].T, np.float32)),
            "wot": wot_bf,
            "csa": csa, "csb": csb, "cmask": cmask,
            "gate2": gate_f[c * n_local:(c + 1) * n_local].reshape(1, n_local),
        })
    return in_maps


def assemble_output(results, n_cores=8, B=2, S=2048, D=2048):
    TPC2 = S // n_cores
    y = np.zeros((B, S, D), np.float32)
    for c in range(n_cores):
        yc = results[c]["y"]
        for b in range(B):
            y[b, TPC2 * c:TPC2 * (c + 1), :] = yc[b * TPC2:(b + 1) * TPC2]
    return y


# ---------------------------------------------------------------------------
# Harness entry point: takes FULL inputs, returns FULL output.
# ---------------------------------------------------------------------------

def kernel(x, adapter, wq, wk, wv, wo, gate, freqs_cis, mask):
    """Gated-adapter attention on 8 TRN2 NeuronCores (head-sharded TP).

    probes="internalq" keeps one tail probe DMA of qt_s in the graph (its
    end-of-graph read pins the tensor's SBUF liveness, which steers the
    allocator/scheduler away from a hazard that a fully stripped build
    deterministically exposes) but writes it to on-device DRAM scratch
    instead of an ExternalOutput, so it costs no host transfer.
    """
    from concourse.bass_utils import run_bass_kernel_spmd

    n_cores = 8
    nc = build_attn(n_cores=n_cores, probes="internalq")
    in_maps = make_in_maps(x, adapter, wq, wk, wv, wo, gate, freqs_cis,
                           n_cores=n_cores)
    r = run_bass_kernel_spmd(nc, in_maps, core_ids=list(range(n_cores)))
    return assemble_output(r.results, n_cores=n_cores)

